# revision 4
# baseline (speedup 1.0000x reference)
"""Trainium2 Bass kernel for nn_CorrelationLayer.

Math (derived from the reference conv formulation):
  out[b, 0, i, j] = sum_{c,y,x} feat1[b,c,y+i-2,x+j-2] * feat2[b,c,y,x]
with out-of-range feat1 reads contributing zero. i.e. 16 shifted
dot-products per batch over the (C, H, W) = (512, 4, 4) volume.

Strategy: pure data parallel over batch (8 cores x 512 batches).
Per core, batch goes on SBUF partitions (128 at a time -> perfectly
contiguous 4 MiB HBM reads), and each displacement's multiply+reduce
runs as fused DVE scalar_tensor_tensor (multiply + free-dim sum) per
valid y-row, with a final tensor_reduce folding the y-row partials.
"""

import sys

import numpy as np

sys.path.insert(0, "/opt/trn_rl_repo")

import concourse.bacc as bacc
import concourse.mybir as mybir
import concourse.tile as tile
from concourse import bass_utils

B, C, H, W = 4096, 512, 4, 4
NCORES = 8
BL = B // NCORES          # 512 batches per core
F = C * H * W             # 8192 elements per batch
PT = 128                  # partition tile (batches per SBUF tile)
NT = BL // PT             # 4 batch-tiles per core

_cached_nc = None


def _emit_body(nc, tc, f1d, f2d, outd):
    """Emit one full pass over this core's shard (inside a TileContext)."""
    with (
        tc.tile_pool(name="io", bufs=2) as iop,
        tc.tile_pool(name="sc", bufs=1) as scp,
        tc.tile_pool(name="ac", bufs=2) as acp,
    ):
        for t in range(NT):
            t1 = iop.tile([PT, F], mybir.dt.float32, tag="t1", name="t1")
            t2 = iop.tile([PT, F], mybir.dt.float32, tag="t2", name="t2")
            nc.sync.dma_start(out=t1[:], in_=f1d[t * PT:(t + 1) * PT, :])
            nc.sync.dma_start(out=t2[:], in_=f2d[t * PT:(t + 1) * PT, :])
            prod = scp.tile([PT, F], mybir.dt.float32, tag="prod", name="prod")
            # per-(displacement, y-row) partial sums, padded to 4 rows
            acc = acp.tile([PT, 64], mybir.dt.float32, tag="acc", name="acc")
            fin = acp.tile([PT, 16], mybir.dt.float32, tag="fin", name="fin")
            nc.vector.memset(acc[:], 0.0)

            a1 = t1.rearrange("p (c y x) -> p c y x", y=H, x=W)
            a2 = t2.rearrange("p (c y x) -> p c y x", y=H, x=W)
            ap = prod.rearrange("p (c y x) -> p c y x", y=H, x=W)

            # Walrus only accepts 2 free dims on DVE ops, so each
            # displacement is split into its h y-rows ([c, x-window] APs);
            # scalar_tensor_tensor fuses multiply + free-dim reduce.
            for i in range(4):
                for j in range(4):
                    di, dj = i - 2, j - 2
                    y0, y1 = max(0, -di), min(H - 1, H - 1 - di)
                    x0, x1 = max(0, -dj), min(W - 1, W - 1 - dj)
                    for y in range(y0, y1 + 1):
                        w1 = a1[:, :, y + di, x0 + dj:x1 + 1 + dj]
                        w2 = a2[:, :, y, x0:x1 + 1]
                        po = ap[:, :, y, x0:x1 + 1]
                        s = (i * 4 + j) * 4 + (y - y0)
                        nc.vector.scalar_tensor_tensor(
                            out=po,
                            in0=w1,
                            scalar=1.0,
                            in1=w2,
                            op0=mybir.AluOpType.mult,
                            op1=mybir.AluOpType.mult,
                            accum_out=acc[:, s:s + 1],
                        )
            nc.vector.tensor_reduce(
                out=fin[:],
                in_=acc.rearrange("p (d y) -> p d y", y=4),
                axis=mybir.AxisListType.X,
                op=mybir.AluOpType.add,
            )
            nc.sync.dma_start(out=outd[t * PT:(t + 1) * PT, :], in_=fin[:])


def _build(reps: int = 1):
    nc = bacc.Bacc("TRN2", target_bir_lowering=False, debug=False)
    f1d = nc.dram_tensor("feat1", [BL, F], mybir.dt.float32, kind="ExternalInput").ap()
    f2d = nc.dram_tensor("feat2", [BL, F], mybir.dt.float32, kind="ExternalInput").ap()
    outd = nc.dram_tensor("out", [BL, 16], mybir.dt.float32, kind="ExternalOutput").ap()

    with tile.TileContext(nc) as tc:
        if reps == 1:
            _emit_body(nc, tc, f1d, f2d, outd)
        else:
            with tc.For_i(0, reps, 1):
                _emit_body(nc, tc, f1d, f2d, outd)

    nc.compile()
    return nc


def _get_nc():
    global _cached_nc
    if _cached_nc is None:
        _cached_nc = _build()
    return _cached_nc


def kernel(feat1, feat2):
    f1 = np.ascontiguousarray(np.asarray(feat1, dtype=np.float32)).reshape(B, F)
    f2 = np.ascontiguousarray(np.asarray(feat2, dtype=np.float32)).reshape(B, F)
    nc = _get_nc()
    in_maps = [
        {"feat1": f1[k * BL:(k + 1) * BL], "feat2": f2[k * BL:(k + 1) * BL]}
        for k in range(NCORES)
    ]
    res = bass_utils.run_bass_kernel_spmd(nc, in_maps, list(range(NCORES)))
    out = np.concatenate([res.results[k]["out"] for k in range(NCORES)], axis=0)
    return out.reshape(B, 1, H, W)
